# revision 21
# baseline (speedup 1.0000x reference)
"""Isomorphic feature extraction kernel for Trainium2 (8 NeuronCores).

Math (per batch b, channel c):
  sub[n]    = 5x5 sliding windows of x[b]              (n = 3600 windows)
  A[c,p]    = P_p @ K_c @ P_p^T                        (p = 120 perms)
  q[n,c,p]  = 2<sub,A> - ||A||^2 - ||sub||^2           (= -dist)
  feat[n,c] = max_p q[n,c,p]
  out       = softmax_n(feat)  flattened to (B, n*c)

Device mapping: one core per (batch, channel-half) pair -> 8 cores.

The q matmul is a single fp32r GEMM (contraction K=104) built from exact
bf16 hi/residual splits of both sides; -||A||^2 rides on two ones-rows of
the lhsT and -||sub||^2 on two per-window rows of the lhsT against
ones-columns in the rhs (so no separate bias path is needed).

The 120-way max per (n,c) is the bottleneck: every non-PE engine reads at
~1 elem/cycle/partition, and GpSimd cannot touch PSUM.  So the perms are
paired on the host: rhs columns hold s=(q_a+q_b)/2 and d=(q_a-q_b)/2 per
pair, and max(q_a,q_b) = s + |d|.  Per 128-row tile:
  - PE writes s into one 2-bank psum tile, d into another.
  - Act computes |d| (psum -> SBUF), the only engine besides DVE that can
    read psum and is otherwise idle.
  - F4 tiles: PE accumulates |d| back onto s via an identity matmul
    (psum accumulation), then one DVE reduce_max over the 60 pair-maxes.
  - F5 tiles: DVE adds s+|d| into SBUF and GpSimd (SBUF-only) runs a
    tensor_tensor max tree, freeing DVE cycles.
This splits the reduction load DVE/Act/Pool/PE roughly evenly (~30us
each) instead of 62us on DVE alone.

Softmax: exp on Act per tile; denominator = ones-vector matmul
(partition-dim sum) + small DVE reduce; reciprocal + broadcast multiply.
Pad rows (n in [3600,3712)) carry -25e6 in the -||sub||^2 row so exp
underflows to 0 and they drop out of the denominator.
"""

import numpy as np
import ml_dtypes
from itertools import permutations

import concourse.bacc as bacc
import concourse.mybir as mybir
from concourse import tile
from concourse import bass_utils

F32 = mybir.dt.float32
F32R = mybir.dt.float32r

B = 4
KS = 5
NH = 60
N = NH * NH            # 3600 subgraph windows
NT = 29                # 128-row tiles over n
NPAD = NT * 128        # 3712
C = 32
CH = 16                # channels per core
NPERM = 120
NPAIR = 60             # perm pairs
PL = 30                # pairs per bank
KDIM = 104             # 4 x 25 split-product rows + 2 nA rows + 2 nx rows
BANKF = 512            # fp32 elems per psum bank
COLS = CH * PL         # 480 used columns per bank

_CACHE = {}


def _bf16(a):
    return a.astype(ml_dtypes.bfloat16).astype(np.float32)


def _perm_mats():
    perms = list(permutations(range(KS)))
    P = np.zeros((len(perms), KS, KS), dtype=np.float32)
    for idx, p in enumerate(perms):
        P[idx, np.arange(KS), np.array(p)] = 1.0
    return P


def _col_order(a):
    """(16, 60, 25) -> (25, 960) in column order bank*480 + c*30 + j."""
    return a.reshape(CH, 2, PL, 25).transpose(3, 1, 0, 2).reshape(25, 2 * COLS)


def _row_order(a):
    """(16, 60) -> (960,) same column order."""
    return a.reshape(CH, 2, PL).transpose(1, 0, 2).reshape(-1)


def _make_rhs(kernel1, Pm, h):
    A = np.einsum("pik,ckl,pjl->cpij", Pm, kernel1, Pm)  # (32, 120, 5, 5)
    A = A[h * CH:(h + 1) * CH]                           # (16, 120, 5, 5)
    nA = (A.astype(np.float64) ** 2).sum(axis=(-2, -1)).astype(np.float32)
    Q = 2.0 * A.reshape(CH, NPERM, 25)
    Qa, Qb = Q[:, 0::2], Q[:, 1::2]                      # (16, 60, 25)
    nAa, nAb = nA[:, 0::2], nA[:, 1::2]                  # (16, 60)
    rhs = np.zeros((KDIM, 4 * COLS), dtype=np.float32)
    for half, X, nAx in (
        (0, (Qa + Qb) * 0.5, (nAa + nAb) * 0.5),         # s columns
        (1, (Qa - Qb) * 0.5, (nAa - nAb) * 0.5),         # d columns
    ):
        cols = slice(half * 2 * COLS, (half + 1) * 2 * COLS)
        xh = _bf16(X)
        rhs[0:25, cols] = _col_order(xh)
        rhs[25:50, cols] = rhs[0:25, cols]
        rhs[50:75, cols] = _col_order(X - xh)
        rhs[75:100, cols] = rhs[50:75, cols]
        nh = _bf16(nAx)
        rhs[100, cols] = -_row_order(nh)
        rhs[101, cols] = -_row_order(nAx - nh)
        if half == 0:
            rhs[102, cols] = 1.0   # -||sub||^2 rides on s columns only
            rhs[103, cols] = 1.0
    return rhs


def _im2col(img):
    """(64,64) -> (25, 3712) window rows; pad columns zero."""
    w = np.lib.stride_tricks.sliding_window_view(img, (KS, KS))  # (60,60,5,5)
    out = np.zeros((25, NPAD), dtype=np.float32)
    out[:, :N] = w.reshape(N, 25).T
    return out


def _make_lhs(xb):
    xb = np.ascontiguousarray(xb)
    xbh = _bf16(xb)
    lhs = np.zeros((KDIM, NPAD), dtype=np.float32)
    lhs[0:25] = _im2col(xbh)
    lhs[25:50] = _im2col(xb - xbh)
    lhs[50:75] = lhs[0:25]
    lhs[75:100] = lhs[25:50]
    lhs[100:102, :N] = 1.0                       # ones rows for -||A||^2
    negnx = -_im2col(xb * xb)[:, :N].sum(axis=0)  # -||sub||^2 per window
    nh = _bf16(negnx)
    lhs[102, :N] = nh
    lhs[103, :N] = negnx - nh
    lhs[102, N:] = -25.0e6                       # pad windows: exp -> 0
    return lhs


def _body(nc, tc, lhs, rhs, ident, out):
    ACT = mybir.ActivationFunctionType

    with tc.tile_pool(name="const", bufs=1) as cp:
        L = cp.tile([128, NPAD], F32R)
        R = cp.tile([KDIM, 4 * COLS], F32R)
        I = cp.tile([128, 128], F32R)
        feat = cp.tile([128, NT * CH], F32)
        E = cp.tile([128, NT * CH], F32)
        onescol = cp.tile([128, 1], F32)
        ones1 = cp.tile([1, 128], F32)
        densum = cp.tile([1, CH], F32)
        recip = cp.tile([1, CH], F32)
        outsb = cp.tile([128, NT * CH], F32)
        warm = cp.tile([1, 4], F32)

        # preload the exp/abs LUT set while the DMAs run
        nc.vector.memset(warm[:, :], 0.0)
        nc.scalar.activation(warm[:, :], warm[:, :], ACT.Exp)

        nc.vector.memset(onescol[:, :], 1.0)
        nc.vector.memset(ones1[:, :], 1.0)

        # Spread DMA issuance across three engine queues so the first tile's
        # operands land ~1.5us in: SP walks the lhs chunks, GpSimd brings
        # bank 0/2 of the rhs + identity, Act banks 1/3.
        CHUNK = NPAD // 8
        for k in range(8):
            lo, hi = k * CHUNK, (k + 1) * CHUNK
            nc.sync.dma_start(out=L[0:KDIM, lo:hi], in_=lhs[:, lo:hi])
        for b in (0, 2):
            nc.gpsimd.dma_start(
                out=R[:, b * COLS:(b + 1) * COLS],
                in_=rhs[:, b * COLS:(b + 1) * COLS],
            )
        for b in (1, 3):
            nc.scalar.dma_start(
                out=R[:, b * COLS:(b + 1) * COLS],
                in_=rhs[:, b * COLS:(b + 1) * COLS],
            )
        nc.gpsimd.dma_start(out=I[:, :], in_=ident[:, :])

        # Warm the PE p-state during the DMA wait: ~3us of continuous dummy
        # matmuls brings the tensor engine to full clock before the first
        # real tile (the cost model halves throughput until then).
        pewarm = cp.tile([128, 64], F32)
        nc.vector.memset(pewarm[:, :], 0.0)
        with tc.tile_pool(name="pewarm_ps", bufs=1, space="PSUM") as wp:
            wps = wp.tile([1, 64], F32, tag="w")
            for _ in range(16):
                nc.tensor.matmul(
                    wps[0:1, 0:64],
                    pewarm[:, 0:1],
                    pewarm[:, :],
                )

        def emit_produce(t, sp, dp, zp):
            ps = sp.tile([128, 2 * BANKF], F32, tag="s")
            pd = dp.tile([128, 2 * BANKF], F32, tag="d")
            z = zp.tile([128, 2 * COLS], F32R, tag="z")
            lt = L[0:KDIM, t * 128:(t + 1) * 128]
            for b in range(2):
                nc.tensor.matmul(
                    ps[:, b * BANKF:b * BANKF + COLS],
                    lt,
                    R[:, b * COLS:(b + 1) * COLS],
                    start=True,
                    stop=False,
                    skip_group_check=True,
                )
            for b in range(2):
                nc.tensor.matmul(
                    pd[:, b * BANKF:b * BANKF + COLS],
                    lt,
                    R[:, (2 + b) * COLS:(3 + b) * COLS],
                )
            dv = (
                pd[:, :]
                .rearrange("q (b s) -> q b s", s=BANKF)[:, :, 0:COLS]
            )
            zv = z[:, :].rearrange("q (b s) -> q b s", s=COLS)
            nc.scalar.activation(zv, dv, ACT.Abs)
            return ps, z

        def emit_consume(t, ps, z):
            fo = feat[:, t * CH:(t + 1) * CH]
            sv = (
                ps[:, :]
                .rearrange("q (b s) -> q b s", s=BANKF)[:, :, 0:COLS]
                .rearrange("q b (c p) -> q b c p", p=PL)
                .transpose([0, 2, 1, 3])
            )
            for b in range(2):
                nc.tensor.matmul(
                    ps[:, b * BANKF:b * BANKF + COLS],
                    I[:, :],
                    z[:, b * COLS:(b + 1) * COLS],
                    start=False,
                    stop=True,
                    skip_group_check=True,
                )
            nc.vector.reduce_max(fo, sv, axis=mybir.AxisListType.XY)
            # batch exp over groups of 4 tiles to amortize Act SBUF latency
            if t % 4 == 3 or t == NT - 1:
                lo = (t // 4) * 4
                nc.scalar.activation(
                    E[:, lo * CH:(t + 1) * CH],
                    feat[:, lo * CH:(t + 1) * CH],
                    ACT.Exp,
                )

        with tc.tile_pool(name="psum_s", bufs=2, space="PSUM") as sp, \
             tc.tile_pool(name="psum_d", bufs=2, space="PSUM") as dp, \
             tc.tile_pool(name="zbuf", bufs=3) as zp:
            prev = None
            for t in range(NT):
                cur = emit_produce(t, sp, dp, zp)
                if prev is not None:
                    emit_consume(t - 1, *prev)
                prev = cur
            emit_consume(NT - 1, *prev)

        with tc.tile_pool(name="psum2", bufs=1, space="PSUM") as pp2:
            den = pp2.tile([1, BANKF], F32, tag="den")
            nc.tensor.matmul(den[0:1, 0:NT * CH], onescol[:, :], E[:, :])
            bc = pp2.tile([128, CH], F32, tag="bc")
            dv = den[0:1, 0:NT * CH].rearrange("q (t c) -> q t c", c=CH).transpose([0, 2, 1])
            nc.vector.reduce_sum(densum[:, :], dv, axis=mybir.AxisListType.X)
            nc.vector.reciprocal(recip[:, :], densum[:, :])
            nc.tensor.matmul(bc[:, :], ones1[:, :], recip[:, :])
            Ev = E[:, :].rearrange("q (t c) -> q t c", c=CH)
            bv = bc[:, :].unsqueeze(1).broadcast_to((128, NT, CH))
            ov = outsb[:, :].rearrange("q (t c) -> q t c", c=CH)
            HALF = 15 * CH
            nc.vector.tensor_mul(
                out=ov[:, 0:15], in0=Ev[:, 0:15], in1=bv[:, 0:15]
            )
            nc.sync.dma_start(
                out=out[0:15].transpose([1, 0, 2]), in_=outsb[:, 0:HALF]
            )
            nc.vector.tensor_mul(
                out=ov[:, 15:NT], in0=Ev[:, 15:NT], in1=bv[:, 15:NT]
            )
            nc.sync.dma_start(
                out=out[15:NT].transpose([1, 0, 2]), in_=outsb[:, HALF:NT * CH]
            )


def _build():
    if "nc" in _CACHE:
        return _CACHE["nc"]
    nc = bacc.Bacc("TRN2", target_bir_lowering=False, debug=False, num_devices=8)
    lhs_d = nc.declare_dram_parameter("lhs", [KDIM, NPAD], F32R, isOutput=False)
    rhs_d = nc.declare_dram_parameter("rhs", [KDIM, 4 * COLS], F32R, isOutput=False)
    id_d = nc.declare_dram_parameter("ident", [128, 128], F32R, isOutput=False)
    out_d = nc.declare_dram_parameter("out", [NT, 128, CH], F32, isOutput=True)
    with tile.TileContext(nc) as tc:
        _body(nc, tc, lhs_d.ap(), rhs_d.ap(), id_d.ap(), out_d.ap())
    nc.compile()
    _CACHE["nc"] = nc
    return nc


def make_in_maps(x, kernel1, P):
    x = np.asarray(x, dtype=np.float32)
    kernel1 = np.asarray(kernel1, dtype=np.float32)
    Pm = _perm_mats()
    rhs_halves = [_make_rhs(kernel1, Pm, h) for h in range(2)]
    ident = np.eye(128, dtype=np.float32)
    in_maps = []
    for core in range(8):
        b, h = core // 2, core % 2
        in_maps.append({
            "lhs": _make_lhs(x[b]),
            "rhs": rhs_halves[h],
            "ident": ident,
        })
    return in_maps


def assemble(results):
    full = np.empty((B, N, C), dtype=np.float32)
    for core in range(8):
        b, h = core // 2, core % 2
        o = np.asarray(results[core]["out"]).reshape(NPAD, CH)[:N]
        full[b, :, h * CH:(h + 1) * CH] = o
    return full.reshape(B, -1)


def kernel(x, kernel1, P):
    nc = _build()
    in_maps = make_in_maps(x, kernel1, P)
    res = bass_utils.run_bass_kernel_spmd(nc, in_maps, core_ids=list(range(8)))
    return assemble(res.results)


# revision 22
# speedup vs baseline: 1.0157x; 1.0157x over previous
"""Isomorphic feature extraction kernel for Trainium2 (8 NeuronCores).

Math (per batch b, channel c):
  sub[n]    = 5x5 sliding windows of x[b]              (n = 3600 windows)
  A[c,p]    = P_p @ K_c @ P_p^T                        (p = 120 perms)
  q[n,c,p]  = 2<sub,A> - ||A||^2 - ||sub||^2           (= -dist)
  feat[n,c] = max_p q[n,c,p]
  out       = softmax_n(feat)  flattened to (B, n*c)

Device mapping: one core per (batch, channel-half) pair -> 8 cores.

The q matmul is a single fp32r GEMM (contraction K=104) built from exact
bf16 hi/residual splits of both sides; -||A||^2 rides on two ones-rows of
the lhsT and -||sub||^2 on two per-window rows of the lhsT against
ones-columns in the rhs (so no separate bias path is needed).

The 120-way max per (n,c) is the bottleneck: every non-PE engine reads at
~1 elem/cycle/partition, and GpSimd cannot touch PSUM.  So the perms are
paired on the host: rhs columns hold s=(q_a+q_b)/2 and d=(q_a-q_b)/2 per
pair, and max(q_a,q_b) = s + |d|.  Per 128-row tile:
  - PE writes s into one 2-bank psum tile, d into another.
  - Act computes |d| (psum -> SBUF), the only engine besides DVE that can
    read psum and is otherwise idle.
  - F4 tiles: PE accumulates |d| back onto s via an identity matmul
    (psum accumulation), then one DVE reduce_max over the 60 pair-maxes.
  - F5 tiles: DVE adds s+|d| into SBUF and GpSimd (SBUF-only) runs a
    tensor_tensor max tree, freeing DVE cycles.
This splits the reduction load DVE/Act/Pool/PE roughly evenly (~30us
each) instead of 62us on DVE alone.

Softmax: exp on Act per tile; denominator = ones-vector matmul
(partition-dim sum) + small DVE reduce; reciprocal + broadcast multiply.
Pad rows (n in [3600,3712)) carry -25e6 in the -||sub||^2 row so exp
underflows to 0 and they drop out of the denominator.
"""

import numpy as np
import ml_dtypes
from itertools import permutations

import concourse.bacc as bacc
import concourse.mybir as mybir
from concourse import tile
from concourse import bass_utils

F32 = mybir.dt.float32
F32R = mybir.dt.float32r

B = 4
KS = 5
NH = 60
N = NH * NH            # 3600 subgraph windows
NT = 29                # 128-row tiles over n
NPAD = NT * 128        # 3712
C = 32
CH = 16                # channels per core
NPERM = 120
NPAIR = 60             # perm pairs
PL = 30                # pairs per bank
KDIM = 104             # 4 x 25 split-product rows + 2 nA rows + 2 nx rows
BANKF = 512            # fp32 elems per psum bank
COLS = CH * PL         # 480 used columns per bank

_CACHE = {}


def _bf16(a):
    return a.astype(ml_dtypes.bfloat16).astype(np.float32)


def _perm_mats():
    perms = list(permutations(range(KS)))
    P = np.zeros((len(perms), KS, KS), dtype=np.float32)
    for idx, p in enumerate(perms):
        P[idx, np.arange(KS), np.array(p)] = 1.0
    return P


def _col_order(a):
    """(16, 60, 25) -> (25, 960) in column order bank*480 + c*30 + j."""
    return a.reshape(CH, 2, PL, 25).transpose(3, 1, 0, 2).reshape(25, 2 * COLS)


def _row_order(a):
    """(16, 60) -> (960,) same column order."""
    return a.reshape(CH, 2, PL).transpose(1, 0, 2).reshape(-1)


def _make_rhs(kernel1, Pm, h):
    A = np.einsum("pik,ckl,pjl->cpij", Pm, kernel1, Pm)  # (32, 120, 5, 5)
    A = A[h * CH:(h + 1) * CH]                           # (16, 120, 5, 5)
    nA = (A.astype(np.float64) ** 2).sum(axis=(-2, -1)).astype(np.float32)
    Q = 2.0 * A.reshape(CH, NPERM, 25)
    Qa, Qb = Q[:, 0::2], Q[:, 1::2]                      # (16, 60, 25)
    nAa, nAb = nA[:, 0::2], nA[:, 1::2]                  # (16, 60)
    rhs = np.zeros((KDIM, 4 * COLS), dtype=np.float32)
    for half, X, nAx in (
        (0, (Qa + Qb) * 0.5, (nAa + nAb) * 0.5),         # s columns
        (1, (Qa - Qb) * 0.5, (nAa - nAb) * 0.5),         # d columns
    ):
        cols = slice(half * 2 * COLS, (half + 1) * 2 * COLS)
        xh = _bf16(X)
        rhs[0:25, cols] = _col_order(xh)
        rhs[25:50, cols] = rhs[0:25, cols]
        rhs[50:75, cols] = _col_order(X - xh)
        rhs[75:100, cols] = rhs[50:75, cols]
        nh = _bf16(nAx)
        rhs[100, cols] = -_row_order(nh)
        rhs[101, cols] = -_row_order(nAx - nh)
        if half == 0:
            rhs[102, cols] = 1.0   # -||sub||^2 rides on s columns only
            rhs[103, cols] = 1.0
    return rhs


def _im2col(img):
    """(64,64) -> (25, 3712) window rows; pad columns zero."""
    w = np.lib.stride_tricks.sliding_window_view(img, (KS, KS))  # (60,60,5,5)
    out = np.zeros((25, NPAD), dtype=np.float32)
    out[:, :N] = w.reshape(N, 25).T
    return out


def _make_lhs(xb):
    xb = np.ascontiguousarray(xb)
    xbh = _bf16(xb)
    lhs = np.zeros((KDIM, NPAD), dtype=np.float32)
    lhs[0:25] = _im2col(xbh)
    lhs[25:50] = _im2col(xb - xbh)
    lhs[50:75] = lhs[0:25]
    lhs[75:100] = lhs[25:50]
    lhs[100:102, :N] = 1.0                       # ones rows for -||A||^2
    negnx = -_im2col(xb * xb)[:, :N].sum(axis=0)  # -||sub||^2 per window
    nh = _bf16(negnx)
    lhs[102, :N] = nh
    lhs[103, :N] = negnx - nh
    lhs[102, N:] = -25.0e6                       # pad windows: exp -> 0
    return lhs


def _body(nc, tc, lhs, rhs, ident, out):
    ACT = mybir.ActivationFunctionType

    with tc.tile_pool(name="const", bufs=1) as cp:
        L = cp.tile([128, NPAD], F32R)
        R = cp.tile([KDIM, 4 * COLS], F32R)
        I = cp.tile([128, 128], F32R)
        feat = cp.tile([128, NT * CH], F32)
        E = cp.tile([128, NT * CH], F32)
        onescol = cp.tile([128, 1], F32)
        ones1 = cp.tile([1, 128], F32)
        densum = cp.tile([1, CH], F32)
        recip = cp.tile([1, CH], F32)
        outsb = cp.tile([128, NT * CH], F32)
        warm = cp.tile([1, 4], F32)

        # preload the exp/abs LUT set while the DMAs run
        nc.vector.memset(warm[:, :], 0.0)
        nc.scalar.activation(warm[:, :], warm[:, :], ACT.Exp)

        nc.vector.memset(onescol[:, :], 1.0)
        nc.vector.memset(ones1[:, :], 1.0)

        # Spread DMA issuance across three engine queues so the first tile's
        # operands land ~1.5us in: SP walks the lhs chunks, GpSimd brings
        # bank 0/2 of the rhs + identity, Act banks 1/3.
        CHUNK = NPAD // 8
        for k in range(8):
            lo, hi = k * CHUNK, (k + 1) * CHUNK
            nc.sync.dma_start(out=L[0:KDIM, lo:hi], in_=lhs[:, lo:hi])
        for b in (0, 2):
            nc.gpsimd.dma_start(
                out=R[:, b * COLS:(b + 1) * COLS],
                in_=rhs[:, b * COLS:(b + 1) * COLS],
            )
        for b in (1, 3):
            nc.scalar.dma_start(
                out=R[:, b * COLS:(b + 1) * COLS],
                in_=rhs[:, b * COLS:(b + 1) * COLS],
            )
        nc.gpsimd.dma_start(out=I[:, :], in_=ident[:, :])



        def emit_produce(t, sp, dp, zp):
            ps = sp.tile([128, 2 * BANKF], F32, tag="s")
            pd = dp.tile([128, 2 * BANKF], F32, tag="d")
            z = zp.tile([128, 2 * COLS], F32R, tag="z")
            lt = L[0:KDIM, t * 128:(t + 1) * 128]
            for b in range(2):
                nc.tensor.matmul(
                    ps[:, b * BANKF:b * BANKF + COLS],
                    lt,
                    R[:, b * COLS:(b + 1) * COLS],
                    start=True,
                    stop=False,
                    skip_group_check=True,
                )
            for b in range(2):
                nc.tensor.matmul(
                    pd[:, b * BANKF:b * BANKF + COLS],
                    lt,
                    R[:, (2 + b) * COLS:(3 + b) * COLS],
                )
            dv = (
                pd[:, :]
                .rearrange("q (b s) -> q b s", s=BANKF)[:, :, 0:COLS]
            )
            zv = z[:, :].rearrange("q (b s) -> q b s", s=COLS)
            nc.scalar.activation(zv, dv, ACT.Abs)
            return ps, z

        def emit_consume(t, ps, z):
            fo = feat[:, t * CH:(t + 1) * CH]
            sv = (
                ps[:, :]
                .rearrange("q (b s) -> q b s", s=BANKF)[:, :, 0:COLS]
                .rearrange("q b (c p) -> q b c p", p=PL)
                .transpose([0, 2, 1, 3])
            )
            for b in range(2):
                nc.tensor.matmul(
                    ps[:, b * BANKF:b * BANKF + COLS],
                    I[:, :],
                    z[:, b * COLS:(b + 1) * COLS],
                    start=False,
                    stop=True,
                    skip_group_check=True,
                )
            nc.vector.reduce_max(fo, sv, axis=mybir.AxisListType.XY)
            # batch exp over groups of 4 tiles to amortize Act SBUF latency
            if t % 4 == 3 or t == NT - 1:
                lo = (t // 4) * 4
                nc.scalar.activation(
                    E[:, lo * CH:(t + 1) * CH],
                    feat[:, lo * CH:(t + 1) * CH],
                    ACT.Exp,
                )

        with tc.tile_pool(name="psum_s", bufs=2, space="PSUM") as sp, \
             tc.tile_pool(name="psum_d", bufs=2, space="PSUM") as dp, \
             tc.tile_pool(name="zbuf", bufs=3) as zp:
            prev = None
            for t in range(NT):
                cur = emit_produce(t, sp, dp, zp)
                if prev is not None:
                    emit_consume(t - 1, *prev)
                prev = cur
            emit_consume(NT - 1, *prev)

        with tc.tile_pool(name="psum2", bufs=1, space="PSUM") as pp2:
            den = pp2.tile([1, BANKF], F32, tag="den")
            nc.tensor.matmul(den[0:1, 0:NT * CH], onescol[:, :], E[:, :])
            bc = pp2.tile([128, CH], F32, tag="bc")
            dv = den[0:1, 0:NT * CH].rearrange("q (t c) -> q t c", c=CH).transpose([0, 2, 1])
            nc.vector.reduce_sum(densum[:, :], dv, axis=mybir.AxisListType.X)
            nc.vector.reciprocal(recip[:, :], densum[:, :])
            nc.tensor.matmul(bc[:, :], ones1[:, :], recip[:, :])
            Ev = E[:, :].rearrange("q (t c) -> q t c", c=CH)
            bv = bc[:, :].unsqueeze(1).broadcast_to((128, NT, CH))
            ov = outsb[:, :].rearrange("q (t c) -> q t c", c=CH)
            HALF = 15 * CH
            nc.vector.tensor_mul(
                out=ov[:, 0:15], in0=Ev[:, 0:15], in1=bv[:, 0:15]
            )
            nc.sync.dma_start(
                out=out[0:15].transpose([1, 0, 2]), in_=outsb[:, 0:HALF]
            )
            nc.vector.tensor_mul(
                out=ov[:, 15:NT], in0=Ev[:, 15:NT], in1=bv[:, 15:NT]
            )
            nc.sync.dma_start(
                out=out[15:NT].transpose([1, 0, 2]), in_=outsb[:, HALF:NT * CH]
            )


def _build():
    if "nc" in _CACHE:
        return _CACHE["nc"]
    nc = bacc.Bacc("TRN2", target_bir_lowering=False, debug=False, num_devices=8)
    lhs_d = nc.declare_dram_parameter("lhs", [KDIM, NPAD], F32R, isOutput=False)
    rhs_d = nc.declare_dram_parameter("rhs", [KDIM, 4 * COLS], F32R, isOutput=False)
    id_d = nc.declare_dram_parameter("ident", [128, 128], F32R, isOutput=False)
    out_d = nc.declare_dram_parameter("out", [NT, 128, CH], F32, isOutput=True)
    with tile.TileContext(nc) as tc:
        _body(nc, tc, lhs_d.ap(), rhs_d.ap(), id_d.ap(), out_d.ap())
    nc.compile()
    _CACHE["nc"] = nc
    return nc


def make_in_maps(x, kernel1, P):
    x = np.asarray(x, dtype=np.float32)
    kernel1 = np.asarray(kernel1, dtype=np.float32)
    Pm = _perm_mats()
    rhs_halves = [_make_rhs(kernel1, Pm, h) for h in range(2)]
    ident = np.eye(128, dtype=np.float32)
    in_maps = []
    for core in range(8):
        b, h = core // 2, core % 2
        in_maps.append({
            "lhs": _make_lhs(x[b]),
            "rhs": rhs_halves[h],
            "ident": ident,
        })
    return in_maps


def assemble(results):
    full = np.empty((B, N, C), dtype=np.float32)
    for core in range(8):
        b, h = core // 2, core % 2
        o = np.asarray(results[core]["out"]).reshape(NPAD, CH)[:N]
        full[b, :, h * CH:(h + 1) * CH] = o
    return full.reshape(B, -1)


def kernel(x, kernel1, P):
    nc = _build()
    in_maps = make_in_maps(x, kernel1, P)
    res = bass_utils.run_bass_kernel_spmd(nc, in_maps, core_ids=list(range(8)))
    return assemble(res.results)


# revision 24
# speedup vs baseline: 1.0161x; 1.0005x over previous
"""Isomorphic feature extraction kernel for Trainium2 (8 NeuronCores).

Math (per batch b, channel c):
  sub[n]    = 5x5 sliding windows of x[b]              (n = 3600 windows)
  A[c,p]    = P_p @ K_c @ P_p^T                        (p = 120 perms)
  q[n,c,p]  = 2<sub,A> - ||A||^2 - ||sub||^2           (= -dist)
  feat[n,c] = max_p q[n,c,p]
  out       = softmax_n(feat)  flattened to (B, n*c)

Device mapping: one core per (batch, channel-half) pair -> 8 cores.

The q matmul is a single fp32r GEMM (contraction K=104) built from exact
bf16 hi/residual splits of both sides; -||A||^2 rides on two ones-rows of
the lhsT and -||sub||^2 on two per-window rows of the lhsT against
ones-columns in the rhs (so no separate bias path is needed).

The 120-way max per (n,c) is the bottleneck: every non-PE engine reads at
~1 elem/cycle/partition, and GpSimd cannot touch PSUM.  So the perms are
paired on the host: rhs columns hold s=(q_a+q_b)/2 and d=(q_a-q_b)/2 per
pair, and max(q_a,q_b) = s + |d|.  Per 128-row tile:
  - PE writes s into one 2-bank psum tile, d into another.
  - Act computes |d| (psum -> SBUF), the only engine besides DVE that can
    read psum and is otherwise idle.
  - F4 tiles: PE accumulates |d| back onto s via an identity matmul
    (psum accumulation), then one DVE reduce_max over the 60 pair-maxes.
  - F5 tiles: DVE adds s+|d| into SBUF and GpSimd (SBUF-only) runs a
    tensor_tensor max tree, freeing DVE cycles.
This splits the reduction load DVE/Act/Pool/PE roughly evenly (~30us
each) instead of 62us on DVE alone.

Softmax: exp on Act per tile; denominator = ones-vector matmul
(partition-dim sum) + small DVE reduce; reciprocal + broadcast multiply.
Pad rows (n in [3600,3712)) carry -25e6 in the -||sub||^2 row so exp
underflows to 0 and they drop out of the denominator.
"""

import numpy as np
import ml_dtypes
from itertools import permutations

import concourse.bacc as bacc
import concourse.mybir as mybir
from concourse import tile
from concourse import bass_utils

F32 = mybir.dt.float32
F32R = mybir.dt.float32r

B = 4
KS = 5
NH = 60
N = NH * NH            # 3600 subgraph windows
NT = 29                # 128-row tiles over n
NPAD = NT * 128        # 3712
C = 32
CH = 16                # channels per core
NPERM = 120
NPAIR = 60             # perm pairs
PL = 30                # pairs per bank
KDIM = 104             # 4 x 25 split-product rows + 2 nA rows + 2 nx rows
BANKF = 512            # fp32 elems per psum bank
COLS = CH * PL         # 480 used columns per bank

_CACHE = {}


def _bf16(a):
    return a.astype(ml_dtypes.bfloat16).astype(np.float32)


def _perm_mats():
    perms = list(permutations(range(KS)))
    P = np.zeros((len(perms), KS, KS), dtype=np.float32)
    for idx, p in enumerate(perms):
        P[idx, np.arange(KS), np.array(p)] = 1.0
    return P


def _col_order(a):
    """(16, 60, 25) -> (25, 960) in column order bank*480 + c*30 + j."""
    return a.reshape(CH, 2, PL, 25).transpose(3, 1, 0, 2).reshape(25, 2 * COLS)


def _row_order(a):
    """(16, 60) -> (960,) same column order."""
    return a.reshape(CH, 2, PL).transpose(1, 0, 2).reshape(-1)


def _make_rhs(kernel1, Pm, h):
    A = np.einsum("pik,ckl,pjl->cpij", Pm, kernel1, Pm)  # (32, 120, 5, 5)
    A = A[h * CH:(h + 1) * CH]                           # (16, 120, 5, 5)
    nA = (A.astype(np.float64) ** 2).sum(axis=(-2, -1)).astype(np.float32)
    Q = 2.0 * A.reshape(CH, NPERM, 25)
    Qa, Qb = Q[:, 0::2], Q[:, 1::2]                      # (16, 60, 25)
    nAa, nAb = nA[:, 0::2], nA[:, 1::2]                  # (16, 60)
    rhs = np.zeros((KDIM, 4 * COLS), dtype=np.float32)
    for half, X, nAx in (
        (0, (Qa + Qb) * 0.5, (nAa + nAb) * 0.5),         # s columns
        (1, (Qa - Qb) * 0.5, (nAa - nAb) * 0.5),         # d columns
    ):
        cols = slice(half * 2 * COLS, (half + 1) * 2 * COLS)
        xh = _bf16(X)
        rhs[0:25, cols] = _col_order(xh)
        rhs[25:50, cols] = rhs[0:25, cols]
        rhs[50:75, cols] = _col_order(X - xh)
        rhs[75:100, cols] = rhs[50:75, cols]
        nh = _bf16(nAx)
        rhs[100, cols] = -_row_order(nh)
        rhs[101, cols] = -_row_order(nAx - nh)
        if half == 0:
            rhs[102, cols] = 1.0   # -||sub||^2 rides on s columns only
            rhs[103, cols] = 1.0
    return rhs


def _im2col(img):
    """(64,64) -> (25, 3712) window rows; pad columns zero."""
    w = np.lib.stride_tricks.sliding_window_view(img, (KS, KS))  # (60,60,5,5)
    out = np.zeros((25, NPAD), dtype=np.float32)
    out[:, :N] = w.reshape(N, 25).T
    return out


def _make_lhs(xb):
    xb = np.ascontiguousarray(xb)
    xbh = _bf16(xb)
    lhs = np.zeros((KDIM, NPAD), dtype=np.float32)
    lhs[0:25] = _im2col(xbh)
    lhs[25:50] = _im2col(xb - xbh)
    lhs[50:75] = lhs[0:25]
    lhs[75:100] = lhs[25:50]
    lhs[100:102, :N] = 1.0                       # ones rows for -||A||^2
    negnx = -_im2col(xb * xb)[:, :N].sum(axis=0)  # -||sub||^2 per window
    nh = _bf16(negnx)
    lhs[102, :N] = nh
    lhs[103, :N] = negnx - nh
    lhs[102, N:] = -25.0e6                       # pad windows: exp -> 0
    return lhs


def _body(nc, tc, lhs, rhs, ident, out):
    ACT = mybir.ActivationFunctionType

    with tc.tile_pool(name="const", bufs=1) as cp:
        L = cp.tile([128, NPAD], F32R)
        R = cp.tile([KDIM, 4 * COLS], F32R)
        I = cp.tile([128, 128], F32R)
        feat = cp.tile([128, NT * CH], F32)
        E = cp.tile([128, NT * CH], F32)
        onescol = cp.tile([128, 1], F32)
        ones1 = cp.tile([1, 128], F32)
        densum = cp.tile([1, CH], F32)
        recip = cp.tile([1, CH], F32)
        outsb = cp.tile([128, NT * CH], F32)
        warm = cp.tile([1, 4], F32)

        # preload the exp/abs LUT set while the DMAs run
        nc.vector.memset(warm[:, :], 0.0)
        nc.scalar.activation(warm[:, :], warm[:, :], ACT.Exp)

        nc.vector.memset(onescol[:, :], 1.0)
        nc.vector.memset(ones1[:, :], 1.0)

        # Spread DMA issuance across three engine queues so the first tile's
        # operands land ~1.5us in: SP walks the lhs chunks, GpSimd brings
        # bank 0/2 of the rhs + identity, Act banks 1/3.
        CHUNK = NPAD // 8
        for k in range(8):
            lo, hi = k * CHUNK, (k + 1) * CHUNK
            nc.sync.dma_start(out=L[0:KDIM, lo:hi], in_=lhs[:, lo:hi])
        for b in (0, 2):
            nc.gpsimd.dma_start(
                out=R[:, b * COLS:(b + 1) * COLS],
                in_=rhs[:, b * COLS:(b + 1) * COLS],
            )
        for b in (1, 3):
            nc.scalar.dma_start(
                out=R[:, b * COLS:(b + 1) * COLS],
                in_=rhs[:, b * COLS:(b + 1) * COLS],
            )
        nc.gpsimd.dma_start(out=I[:, :], in_=ident[:, :])



        def emit_produce(t, sp, dp, zp, pre=None):
            if pre is None:
                ps = sp.tile([128, 2 * BANKF], F32, tag="s")
                pd = dp.tile([128, 2 * BANKF], F32, tag="d")
            else:
                ps, pd = pre
            z = zp.tile([128, 2 * COLS], F32R, tag="z")
            lt = L[0:KDIM, t * 128:(t + 1) * 128]
            for b in range(2):
                nc.tensor.matmul(
                    ps[:, b * BANKF:b * BANKF + COLS],
                    lt,
                    R[:, b * COLS:(b + 1) * COLS],
                    start=True,
                    stop=False,
                    skip_group_check=True,
                )
            for b in range(2):
                nc.tensor.matmul(
                    pd[:, b * BANKF:b * BANKF + COLS],
                    lt,
                    R[:, (2 + b) * COLS:(3 + b) * COLS],
                )
            dv = (
                pd[:, :]
                .rearrange("q (b s) -> q b s", s=BANKF)[:, :, 0:COLS]
            )
            zv = z[:, :].rearrange("q (b s) -> q b s", s=COLS)
            nc.scalar.activation(zv, dv, ACT.Abs)
            return ps, z

        def emit_consume(t, ps, z):
            fo = feat[:, t * CH:(t + 1) * CH]
            sv = (
                ps[:, :]
                .rearrange("q (b s) -> q b s", s=BANKF)[:, :, 0:COLS]
                .rearrange("q b (c p) -> q b c p", p=PL)
                .transpose([0, 2, 1, 3])
            )
            for b in range(2):
                nc.tensor.matmul(
                    ps[:, b * BANKF:b * BANKF + COLS],
                    I[:, :],
                    z[:, b * COLS:(b + 1) * COLS],
                    start=False,
                    stop=True,
                    skip_group_check=True,
                )
            nc.vector.reduce_max(fo, sv, axis=mybir.AxisListType.XY)
            # batch exp over groups of 4 tiles to amortize Act SBUF latency
            if t % 4 == 3 or t == NT - 1:
                lo = (t // 4) * 4
                nc.scalar.activation(
                    E[:, lo * CH:(t + 1) * CH],
                    feat[:, lo * CH:(t + 1) * CH],
                    ACT.Exp,
                )

        with tc.tile_pool(name="psum_s", bufs=2, space="PSUM") as sp, \
             tc.tile_pool(name="psum_d", bufs=2, space="PSUM") as dp, \
             tc.tile_pool(name="zbuf", bufs=3) as zp:
            # Warm the PE p-state during the DMA wait with dummy matmuls into
            # the pad columns of tile 0's s-psum: the cost model runs the PE
            # at half clock until it has been continuously busy for 3us, so
            # an idle-started first tile pays ~2x on its matmuls.
            ps0 = sp.tile([128, 2 * BANKF], F32, tag="s")
            pd0 = dp.tile([128, 2 * BANKF], F32, tag="d")
            pewarm = cp.tile([128, 64], F32)
            nc.vector.memset(pewarm[:, :], 0.0)
            for _ in range(18):
                nc.tensor.matmul(
                    ps0[0:1, 2 * BANKF - 32:2 * BANKF],
                    pewarm[:, 0:1],
                    pewarm[:, 0:32],
                )
            prev = None
            for t in range(NT):
                cur = emit_produce(t, sp, dp, zp, pre=(ps0, pd0) if t == 0 else None)
                if prev is not None:
                    emit_consume(t - 1, *prev)
                prev = cur
            emit_consume(NT - 1, *prev)

        with tc.tile_pool(name="psum2", bufs=1, space="PSUM") as pp2:
            den = pp2.tile([1, BANKF], F32, tag="den")
            nc.tensor.matmul(den[0:1, 0:NT * CH], onescol[:, :], E[:, :])
            bc = pp2.tile([128, CH], F32, tag="bc")
            dv = den[0:1, 0:NT * CH].rearrange("q (t c) -> q t c", c=CH).transpose([0, 2, 1])
            nc.vector.reduce_sum(densum[:, :], dv, axis=mybir.AxisListType.X)
            nc.vector.reciprocal(recip[:, :], densum[:, :])
            nc.tensor.matmul(bc[:, :], ones1[:, :], recip[:, :])
            Ev = E[:, :].rearrange("q (t c) -> q t c", c=CH)
            bv = bc[:, :].unsqueeze(1).broadcast_to((128, NT, CH))
            ov = outsb[:, :].rearrange("q (t c) -> q t c", c=CH)
            HALF = 15 * CH
            nc.vector.tensor_mul(
                out=ov[:, 0:15], in0=Ev[:, 0:15], in1=bv[:, 0:15]
            )
            nc.sync.dma_start(
                out=out[0:15].transpose([1, 0, 2]), in_=outsb[:, 0:HALF]
            )
            nc.vector.tensor_mul(
                out=ov[:, 15:NT], in0=Ev[:, 15:NT], in1=bv[:, 15:NT]
            )
            nc.sync.dma_start(
                out=out[15:NT].transpose([1, 0, 2]), in_=outsb[:, HALF:NT * CH]
            )


def _build():
    if "nc" in _CACHE:
        return _CACHE["nc"]
    nc = bacc.Bacc("TRN2", target_bir_lowering=False, debug=False, num_devices=8)
    lhs_d = nc.declare_dram_parameter("lhs", [KDIM, NPAD], F32R, isOutput=False)
    rhs_d = nc.declare_dram_parameter("rhs", [KDIM, 4 * COLS], F32R, isOutput=False)
    id_d = nc.declare_dram_parameter("ident", [128, 128], F32R, isOutput=False)
    out_d = nc.declare_dram_parameter("out", [NT, 128, CH], F32, isOutput=True)
    with tile.TileContext(nc) as tc:
        _body(nc, tc, lhs_d.ap(), rhs_d.ap(), id_d.ap(), out_d.ap())
    nc.compile()
    _CACHE["nc"] = nc
    return nc


def make_in_maps(x, kernel1, P):
    x = np.asarray(x, dtype=np.float32)
    kernel1 = np.asarray(kernel1, dtype=np.float32)
    Pm = _perm_mats()
    rhs_halves = [_make_rhs(kernel1, Pm, h) for h in range(2)]
    ident = np.eye(128, dtype=np.float32)
    in_maps = []
    for core in range(8):
        b, h = core // 2, core % 2
        in_maps.append({
            "lhs": _make_lhs(x[b]),
            "rhs": rhs_halves[h],
            "ident": ident,
        })
    return in_maps


def assemble(results):
    full = np.empty((B, N, C), dtype=np.float32)
    for core in range(8):
        b, h = core // 2, core % 2
        o = np.asarray(results[core]["out"]).reshape(NPAD, CH)[:N]
        full[b, :, h * CH:(h + 1) * CH] = o
    return full.reshape(B, -1)


def kernel(x, kernel1, P):
    nc = _build()
    in_maps = make_in_maps(x, kernel1, P)
    res = bass_utils.run_bass_kernel_spmd(nc, in_maps, core_ids=list(range(8)))
    return assemble(res.results)
